# revision 42
# baseline (speedup 1.0000x reference)
"""Trainium2 Bass kernel for DecoderAttention (b=2, n=2048, m=1024, d=1024, h=16).

Sharding: 8 cores = 2 (batch) x 4 (head groups of 4 heads).  Each core:
  - projects q/k/v for its 4 heads from x|context (pre-transposed on host),
  - runs causal flash attention in scores-transposed layout [kj, qi]
    (softmax without max subtraction -- scores are bounded; causally masked
    entries multiply to exactly 0 after exp, matching exp(-50000)),
  - computes its partial out-projection  attn_out_g @ Wo[rows_g]  [2048, 1024].
Host sums the 4 head-group partials per batch (the "all-reduce") and adds bo.

All matmuls run in bf16 with f32 PSUM accumulation (validated ~0.4% rel err).

v2 schedule notes (HW-profile driven):
  - The ACT engine's 144 exp activations (~155us busy) are the pacing
    resource; everything else is emitted as filler between score rounds so
    ACT never starves and the PE stays dense (HAM stays at K=8/8).
  - The [1,512] DVE RECIPROCAL (3.3us each, strict-FIFO head-of-line) is
    replaced by one reciprocal_approx_fast per head-pair.
  - Projection PSUM pool has 2 slots so proj groups pipeline through DVE
    evictions; out-projection of chunk c is deferred into chunk c+1's
    rounds.
  - Diagonal score tiles skip the causally-dead query columns in both the
    QK matmul and the exp (the mask multiply re-zeroes the stale region).
  - Input DMAs are sliced and ordered so chunk-0 dependencies land first;
    a dummy exp preloads the ACT spline table during the DMA window.
"""

import os

# The neuron/axon jax backend must be discoverable for the PJRT execution
# path; a JAX_PLATFORMS=cpu pin (used when running the jax reference) would
# hide the trn2 devices from this process.
if os.environ.get("JAX_PLATFORMS", "").strip().lower() == "cpu":
    del os.environ["JAX_PLATFORMS"]

from contextlib import ExitStack

import ml_dtypes
import numpy as np

import concourse.bass as bass
import concourse.tile as tile
from concourse import bacc, mybir
from concourse.bass_utils import run_bass_kernel_spmd

B, N, M, D = 2, 2048, 1024, 1024
H, DH = 16, 64
NM = N + M          # 3072 keys (self + context)
GROUPS = 4          # head groups; 4 heads = 256 cols per group
GC = 256            # columns per head group
NCORES = 8
SCALE = DH ** -0.5
P = 128
KT = D // P         # 8 contraction tiles over d
QCH = 512           # query-chunk width
NQC = N // QCH      # 4 query chunks
NKJ = NM // P       # 24 key tiles
NSELF = N // P      # 16 self key tiles
PTS = 8             # pt ring slots
MASK_DVE = os.environ.get("BASS_MASK_DVE", "") == "1"
FP32 = mybir.dt.float32
BF16 = mybir.dt.bfloat16
BF16NP = ml_dtypes.bfloat16


def _active_kj(c):
    """Key tiles with any unmasked entry for query chunk c (512 queries).

    Chunk 0 runs its (diagonal) self tiles first since the cross columns
    arrive later over DMA; later chunks run cross tiles first so the
    chunk's own self-k/v projections can be produced as same-segment
    fillers instead of loading the previous (PE-saturated) chunk."""
    if c == 0:
        return list(range(0, 4)) + list(range(NSELF, NKJ))
    return list(range(NSELF, NKJ)) + list(range(0, 4 * c + 4))


def _build_module(biased: bool):
    nc = bacc.Bacc(
        "TRN2",
        target_bir_lowering=False,
        debug=False,
        enable_asserts=False,
        num_devices=NCORES,
    )
    xkvT_d = nc.dram_tensor("xkvT", [D, NM], BF16, kind="ExternalInput").ap()
    wq_d = nc.dram_tensor("wq", [D, GC], BF16, kind="ExternalInput").ap()
    wk_d = nc.dram_tensor("wk", [D, GC], BF16, kind="ExternalInput").ap()
    wv_d = nc.dram_tensor("wv", [D, GC], BF16, kind="ExternalInput").ap()
    wo_d = nc.dram_tensor("wo", [GC, D], BF16, kind="ExternalInput").ap()
    msk_d = nc.dram_tensor("msk", [4 * P, QCH], BF16, kind="ExternalInput").ap()
    if biased:
        bq_d = nc.dram_tensor("bq", [1, GC], BF16, kind="ExternalInput").ap()
        bk_d = nc.dram_tensor("bk", [1, GC], BF16, kind="ExternalInput").ap()
        bv_d = nc.dram_tensor("bv", [1, GC], BF16, kind="ExternalInput").ap()
    out_d = nc.dram_tensor("out", [N, D], BF16, kind="ExternalOutput").ap()

    with tile.TileContext(nc) as tc, ExitStack() as ctx:
        const = ctx.enter_context(tc.tile_pool(name="const", bufs=1))
        bcp = ctx.enter_context(tc.tile_pool(name="bcp", bufs=3))
        osbp = ctx.enter_context(tc.tile_pool(name="osbp", bufs=3))
        # PSUM budget: 8 banks = proj/psb(2) + scores(2x2) + av(2)
        ps_main = ctx.enter_context(tc.tile_pool(name="ps_main", bufs=2, space="PSUM"))
        ps_s = ctx.enter_context(tc.tile_pool(name="ps_s", bufs=2, space="PSUM"))
        ps_av = ctx.enter_context(tc.tile_pool(name="ps_av", bufs=2, space="PSUM"))

        # ---- persistent SBUF tensors (column-concatenated k-tiles) ----
        xk = const.tile([P, KT * NM], BF16)          # xkvT: 8 tiles of [128, 3072]
        wqs = const.tile([P, KT * GC], BF16)
        wks = const.tile([P, KT * GC], BF16)
        wvs = const.tile([P, KT * GC], BF16)
        wos = const.tile([P, 2 * D], BF16)           # Wo rows: 2 tiles of [128, 1024]
        mks = const.tile([P, 4 * QCH], BF16)         # 4 diagonal mask tiles
        qT = const.tile([P, 2 * N], BF16)            # [head-pair cols, qi]
        kT = const.tile([P, 2 * NM], BF16)           # [head-pair cols, kj]
        vv = const.tile([P, NKJ * 4 * 65], BF16)     # per kj tile: 4x [v(64)|1]
        aT = const.tile([P, 2 * N], BF16)            # attn_out^T, 2 k-tiles
        ptr = const.tile([P, PTS * 2 * QCH], BF16)   # pt ring (exp'd scores)
        ones_l = const.tile([33, 64], BF16)
        dummy = const.tile([1, 2], FP32)
        if biased:
            bq_s = const.tile([1, GC], BF16)
            bk_s = const.tile([1, GC], BF16)
            bv_s = const.tile([1, GC], BF16)
            ones_row = const.tile([1, QCH], BF16)
            ones_col = const.tile([1, P], BF16)

        # ---- ACT table preload: a dummy exp during the DMA window ----
        nc.vector.memset(dummy[:], 1.0)
        nc.scalar.activation(
            dummy[:, 0:1], dummy[:, 1:2], mybir.ActivationFunctionType.Exp
        )

        # ---- input DMAs, prioritized: chunk-0 pair-0 deps first.
        # One 3D-AP DMA per tensor region (not per kt tile): the sync
        # engine issues descriptors at ~0.5us apiece, so issue count --
        # not bytes -- dominated the old startup critical path. ----
        xk3 = xk.rearrange("p (kt nm) -> p kt nm", kt=KT)
        xs3 = xkvT_d.rearrange("(kt p) nm -> p kt nm", p=P)
        wq3 = wqs.rearrange("p (kt gc) -> p kt gc", kt=KT)
        wk3 = wks.rearrange("p (kt gc) -> p kt gc", kt=KT)
        wv3 = wvs.rearrange("p (kt gc) -> p kt gc", kt=KT)
        wqd3 = wq_d.rearrange("(kt p) gc -> p kt gc", p=P)
        wkd3 = wk_d.rearrange("(kt p) gc -> p kt gc", p=P)
        wvd3 = wv_d.rearrange("(kt p) gc -> p kt gc", p=P)
        # Four parallel DGE queues (one per issuing engine): SP carries the
        # chunk-0 q/k critical path, DVE the cross/context path, Pool the
        # pair-1 weights + chunk-1 columns, ACT the rest.  A single queue
        # is FIFO and measured ~220 GB/s -- splitting both parallelizes
        # the streams and stops a late region blocking an early one.
        nc.sync.dma_start(wq3[:, :, 0:P], wqd3[:, :, 0:P])      # mt0 Wq
        nc.sync.dma_start(xk3[:, :, 0:QCH], xs3[:, :, 0:QCH])   # self cols, chunk 0
        nc.sync.dma_start(wk3[:, :, 0:P], wkd3[:, :, 0:P])      # mt0 Wk
        nc.scalar.dma_start(xk3[:, :, N:N + QCH], xs3[:, :, N:N + QCH])
        nc.scalar.dma_start(xk3[:, :, N + QCH:NM], xs3[:, :, N + QCH:NM])
        nc.scalar.dma_start(wv3[:], wvd3[:])
        nc.gpsimd.dma_start(
            mks.rearrange("p (t q) -> p t q", t=4),
            msk_d.rearrange("(t p) q -> p t q", p=P),
        )
        nc.gpsimd.dma_start(wq3[:, :, P:GC], wqd3[:, :, P:GC])  # mt1 Wq
        nc.gpsimd.dma_start(wk3[:, :, P:GC], wkd3[:, :, P:GC])  # mt1 Wk
        nc.gpsimd.dma_start(xk3[:, :, QCH:2 * QCH], xs3[:, :, QCH:2 * QCH])
        nc.sync.dma_start(xk3[:, :, 2 * QCH:3 * QCH], xs3[:, :, 2 * QCH:3 * QCH])
        nc.gpsimd.dma_start(
            wos.rearrange("p (kt d) -> p kt d", kt=2),
            wo_d.rearrange("(kt p) d -> p kt d", p=P),
        )
        nc.sync.dma_start(xk3[:, :, 3 * QCH:N], xs3[:, :, 3 * QCH:N])
        nc.vector.memset(ones_l[:], 1.0)
        if biased:
            nc.sync.dma_start(bq_s[:], bq_d[:])
            nc.sync.dma_start(bk_s[:], bk_d[:])
            nc.sync.dma_start(bv_s[:], bv_d[:])
            nc.vector.memset(ones_row[:], 1.0)
            nc.vector.memset(ones_col[:], 1.0)
        # pt ring must hold finite values before its first masked use
        for s in range(PTS):
            nc.vector.memset(ptr[:, s * 2 * QCH:(s + 1) * 2 * QCH], 0.0)
        # ones columns interleaved into vv: col (t*260 + h*65 + 64)
        nc.gpsimd.memset(
            vv.rearrange("p (t h x) -> p t h x", t=NKJ, h=4)[:, :, :, 64:65], 1.0
        )

        # ---- emission helpers ----
        def emit_qT_group(mt, c):
            psq = ps_main.tile([P, QCH], FP32, tag="proj", name="psq")
            for kt in range(KT):
                nc.tensor.matmul(
                    psq[:],
                    lhsT=wqs[:, kt * GC + mt * P: kt * GC + (mt + 1) * P],
                    rhs=xk[:, kt * NM + c * QCH: kt * NM + (c + 1) * QCH],
                    start=(kt == 0),
                    stop=(kt == KT - 1) and not biased,
                )
            if biased:
                nc.tensor.matmul(
                    psq[:], lhsT=bq_s[:, mt * P:(mt + 1) * P], rhs=ones_row[:],
                    start=False, stop=True,
                )
            nc.vector.tensor_copy(
                qT[:, mt * N + c * QCH: mt * N + (c + 1) * QCH], psq[:]
            )

        def emit_kT_group(mt, c2):
            psk = ps_main.tile([P, QCH], FP32, tag="proj", name="psk")
            for kt in range(KT):
                nc.tensor.matmul(
                    psk[:],
                    lhsT=wks[:, kt * GC + mt * P: kt * GC + (mt + 1) * P],
                    rhs=xk[:, kt * NM + c2 * QCH: kt * NM + (c2 + 1) * QCH],
                    start=(kt == 0),
                    stop=(kt == KT - 1) and not biased,
                )
            if biased:
                nc.tensor.matmul(
                    psk[:], lhsT=bk_s[:, mt * P:(mt + 1) * P], rhs=ones_row[:],
                    start=False, stop=True,
                )
            nc.vector.tensor_copy(
                kT[:, mt * NM + c2 * QCH: mt * NM + (c2 + 1) * QCH], psk[:]
            )

        def emit_v_group(t):
            psv = ps_main.tile([P, GC], FP32, tag="proj", name="psv")
            for kt in range(KT):
                nc.tensor.matmul(
                    psv[:],
                    lhsT=xk[:, kt * NM + t * P: kt * NM + (t + 1) * P],
                    rhs=wvs[:, kt * GC:(kt + 1) * GC],
                    start=(kt == 0),
                    stop=(kt == KT - 1) and not biased,
                )
            if biased:
                nc.tensor.matmul(
                    psv[:], lhsT=ones_col[:], rhs=bv_s[:], start=False, stop=True,
                )
            nc.vector.tensor_copy(
                vv[:, t * 260:(t + 1) * 260].rearrange("p (h x) -> p h x", h=4)[
                    :, :, 0:64
                ],
                psv.rearrange("p (h x) -> p h x", h=4),
            )

        def emit_outproj_unit(c, it, nh):
            pso = ps_main.tile([P, QCH], FP32, tag="proj", name="pso")
            for kt in range(2):
                nc.tensor.matmul(
                    pso[:],
                    lhsT=aT[:, kt * N + it * P: kt * N + (it + 1) * P],
                    rhs=wos[:, kt * D + nh * QCH: kt * D + (nh + 1) * QCH],
                    start=(kt == 0),
                    stop=(kt == 1),
                )
            osb = osbp.tile([P, QCH], BF16, tag="osb", name="osb")
            nc.vector.tensor_copy(osb[:], pso[:])
            nc.sync.dma_start(
                out_d[it * P:(it + 1) * P, nh * QCH:(nh + 1) * QCH], osb[:]
            )

        rot = [0]  # pt ring rotation

        def emit_attention_segment(c, pair, fillers, chunk_ctx, hard=()):
            """One (chunk, head-pair) flash segment with interleaved filler.

            `hard` fillers are (deadline_round, fn): fn EMITS data consumed
            by this segment's own later rounds, so it must be emitted (and
            thus dep-tracked as the writer) before the consuming round --
            an after-the-reader write becomes a WAR hazard and the reader
            deterministically sees uninitialized SBUF.  `fillers` are
            order-free (consumed only by later segments) and are spread
            evenly for scheduler priority."""
            kjs = _active_kj(c)
            last = len(kjs) - 1
            nfill = len(fillers)
            fdone = 0
            hard = list(hard)
            ps_acc = [None, None]
            pending = None  # (pt_slice_by_head, i) exp'd tiles not yet fed to AV

            def do_av(pts, i):
                # NOTE: all members of this accumulation group must keep the
                # SAME output AP -- column-sliced members corrupt the bank's
                # has_written state on real hardware (sim doesn't model it).
                t = kjs[i]
                for hh in range(2):
                    h = pair * 2 + hh
                    nc.tensor.matmul(
                        ps_acc[hh][:],
                        lhsT=vv[:, t * 260 + h * 65: t * 260 + (h + 1) * 65],
                        rhs=pts[hh],
                        start=(i == 0),
                        stop=(i == last),
                    )

            for i, t in enumerate(kjs):
                pss = ps_s.tile([P, 2 * QCH], FP32, tag="s", name="pss")
                for hh in range(2):
                    lo, hi = hh * 64, hh * 64 + 64
                    nc.tensor.matmul(
                        pss[:, hh * QCH:(hh + 1) * QCH],
                        lhsT=kT[lo:hi, pair * NM + t * P: pair * NM + (t + 1) * P],
                        rhs=qT[lo:hi, pair * N + c * QCH: pair * N + (c + 1) * QCH],
                        start=True,
                        stop=True,
                    )
                slot = rot[0] % PTS
                rot[0] += 1
                pt = ptr[:, slot * 2 * QCH:(slot + 1) * 2 * QCH]
                nc.scalar.activation(
                    pt, pss[:], mybir.ActivationFunctionType.Exp
                )
                if 4 * c <= t < 4 * c + 4:  # diagonal tile: causal mask
                    # on the otherwise-idle GPSIMD engine, keeping the DVE
                    # queue short (a long op at the strict-FIFO head delays
                    # every eviction behind it)
                    dt = t - 4 * c
                    eng = nc.vector if MASK_DVE else nc.gpsimd
                    for hh in range(2):
                        eng.tensor_mul(
                            pt[:, hh * QCH:(hh + 1) * QCH],
                            pt[:, hh * QCH:(hh + 1) * QCH],
                            mks[:, dt * QCH:(dt + 1) * QCH],
                        )
                if i == 0:
                    ps_acc[0] = ps_av.tile([65, QCH], FP32, tag="av", name="av0")
                    ps_acc[1] = ps_av.tile([65, QCH], FP32, tag="av", name="av1")
                if pending is not None:
                    do_av(*pending)
                pending = (
                    [pt[:, hh * QCH:(hh + 1) * QCH] for hh in range(2)], i
                )
                # deadline fillers first, then spread the order-free ones
                while hard and hard[0][0] <= i:
                    hard.pop(0)[1]()
                want = (i + 1) * nfill // len(kjs)
                while fdone < want:
                    fillers[fdone]()
                    fdone += 1
            for _, f in hard:
                f()
            do_av(*pending)
            while fdone < nfill:
                fillers[fdone]()
                fdone += 1

            def normalize():
                # evict accumulators fast to free the AV psum slots.  The
                # iterative reciprocal's cost is free-dim-bound, so all 4
                # heads of the chunk share ONE [97,512] reciprocal (heads
                # at partitions 0/32/64/96), run when pair 1 finishes.
                # Broadcast via TensorE in bf16 and scale straight out of
                # PSUM.  Returned as a closure so the caller can defer it
                # below the next segment's first rounds.
                # (reciprocal_approx_fast is numerically broken on HW via
                # this runtime -- keep the stock iterative reciprocal.)
                if "den4" not in chunk_ctx:
                    chunk_ctx["den4"] = bcp.tile(
                        [97, QCH], FP32, tag="den4", bufs=2, name="den4"
                    )
                    # fill the never-read rows between the head seeds so
                    # the batched reciprocal reads defined, finite data
                    nc.vector.memset(chunk_ctx["den4"][:], 1.0)
                den4 = chunk_ctx["den4"]
                for hh in range(2):
                    h = pair * 2 + hh
                    nc.vector.tensor_copy(
                        den4[32 * h:32 * h + 1, :], ps_acc[hh][64:65, :]
                    )
                    unrm = bcp.tile(
                        [64, QCH], BF16, tag="unrm", bufs=5, name="unrm"
                    )
                    nc.vector.tensor_copy(unrm[:], ps_acc[hh][0:64, :])
                    chunk_ctx[("unrm", h)] = unrm
                if pair == 1:
                    rec4 = bcp.tile(
                        [97, QCH], FP32, tag="rec4", bufs=2, name="rec4"
                    )
                    # rows between the 0/32/64/96 seeds are junk; never read
                    nc.vector.reciprocal(rec4[:], den4[:])
                    for h in range(4):
                        recb = bcp.tile([1, QCH], BF16, tag="recb", name="recb")
                        nc.vector.tensor_copy(recb[:], rec4[32 * h:32 * h + 1, :])
                        psb = ps_main.tile([64, QCH], FP32, tag="proj", name="psb")
                        nc.tensor.matmul(
                            psb[:], lhsT=ones_l[0:1, :], rhs=recb[:],
                            start=True, stop=True,
                        )
                        kt2 = h // 2
                        lo = (h % 2) * 64
                        nc.vector.tensor_mul(
                            aT[lo:lo + 64, kt2 * N + c * QCH: kt2 * N + (c + 1) * QCH],
                            chunk_ctx[("unrm", h)][:],
                            psb[:],
                        )

            return normalize

        # ---- startup projections: minimum prefix for chunk-0 pair-0.
        # Everything else is emitted as segment filler so its scheduler
        # priority sits BELOW the score rounds it must not delay. ----
        emit_qT_group(0, 0)
        emit_kT_group(0, 0)
        for t in range(0, 4):
            emit_v_group(t)

        # ---- main stream: attention segments with interleaved filler ----
        def proj_fillers(c):
            f = []
            if c < NQC - 1:
                for mt in range(2):
                    f.append(lambda mt=mt: emit_qT_group(mt, c + 1))
                    f.append(lambda mt=mt: emit_kT_group(mt, c + 1))
                for t in range(4 * (c + 1), 4 * (c + 1) + 4):
                    f.append(lambda t=t: emit_v_group(t))
            return f

        def outproj_fillers(c):
            f = []
            for it in range(4 * c, 4 * c + 4):
                for nh in range(2):
                    f.append(lambda it=it, nh=nh: emit_outproj_unit(c, it, nh))
            return f

        qg = lambda mt, c: (lambda: emit_qT_group(mt, c))
        kg = lambda mt, c2: (lambda: emit_kT_group(mt, c2))
        vg = lambda t: (lambda: emit_v_group(t))

        # segment (0,0): cross-key/value projections (needed from round 4,
        # in kj order) and pair-1's q/k.  Later chunks run cross-first, so
        # each chunk's own self-k/v projections ride as its own fillers and
        # only q (needed at round 0) must be produced a chunk ahead.
        # chunk-0 kjs = [0..3, 16..23]: cross tile 16+j consumed at round
        # 4+j, its kT group and v tile must be emitted strictly earlier.
        hard00 = (
            [(3, kg(0, 4)), (4, vg(16)), (5, vg(17)), (6, vg(18)),
             (6, kg(0, 5)), (7, vg(19)), (8, vg(20)), (9, vg(21)),
             (10, vg(22)), (11, vg(23))]
        )
        fill00 = [qg(1, 0), kg(1, 0), kg(1, 4), kg(1, 5)]
        cctx = {}
        n0 = emit_attention_segment(0, 0, fill00, cctx, hard=hard00)
        norm_prev = emit_attention_segment(
            0, 1, [n0, qg(0, 1), qg(1, 1)], cctx
        )
        for c in range(1, NQC):
            # chunk-c kjs = [16..23, 0..4c+3]: self tiles 4c+j sit at round
            # 8+4c+j and need this chunk's kT(c)/v groups emitted earlier.
            op = outproj_fillers(c - 1)
            hardA = [(7 + 4 * c, kg(0, c))] + [
                (8 + 4 * c + j, vg(4 * c + j)) for j in range(4)
            ]
            fillA = [norm_prev, kg(1, c)] + op[:4]
            cctx = {}
            n0 = emit_attention_segment(c, 0, fillA, cctx, hard=hardA)
            fillB = [n0] + op[4:]
            if c < NQC - 1:
                fillB += [qg(0, c + 1), qg(1, c + 1)]
            norm_prev = emit_attention_segment(c, 1, fillB, cctx)
        norm_prev()
        for f in outproj_fillers(NQC - 1):
            f()

    nc.compile()
    return nc


_CACHE: dict = {}


def _module(biased: bool):
    if biased not in _CACHE:
        _CACHE[biased] = _build_module(biased)
    return _CACHE[biased]


def _mask_tiles():
    t = np.arange(4)[:, None, None]
    p = np.arange(P)[None, :, None]
    q = np.arange(QCH)[None, None, :]
    return (p + P * t <= q).astype(BF16NP).reshape(4 * P, QCH)


def kernel(x, context, Wq, bq, Wkv, bkv, Wo, bo, mask, context_mask):
    assert bool(np.all(mask)) and bool(np.all(context_mask)), (
        "only all-true padding masks are supported"
    )
    x = np.asarray(x, np.float32)
    context = np.asarray(context, np.float32)
    Wq, bq = np.asarray(Wq, np.float32), np.asarray(bq, np.float32)
    Wkv, bkv = np.asarray(Wkv, np.float32), np.asarray(bkv, np.float32)
    Wo, bo = np.asarray(Wo, np.float32), np.asarray(bo, np.float32)

    biased = bool(np.any(bq) or np.any(bkv))
    nc = _module(biased)

    msk = _mask_tiles()
    xkvT = [
        np.ascontiguousarray(
            np.concatenate([x[b], context[b]], axis=0).T.astype(BF16NP)
        )
        for b in range(B)
    ]
    in_maps = []
    for core in range(NCORES):
        b, g = divmod(core, GROUPS)
        cols = slice(g * GC, (g + 1) * GC)
        im = {
            "xkvT": xkvT[b],
            "wq": (Wq[:, cols] * SCALE).astype(BF16NP),
            "wk": Wkv[:, cols].astype(BF16NP),
            "wv": Wkv[:, D + g * GC: D + (g + 1) * GC].astype(BF16NP),
            "wo": np.ascontiguousarray(Wo[cols, :]).astype(BF16NP),
            "msk": msk,
        }
        if biased:
            im["bq"] = (bq[cols] * SCALE).astype(BF16NP).reshape(1, GC)
            im["bk"] = bkv[cols].astype(BF16NP).reshape(1, GC)
            im["bv"] = bkv[D + g * GC: D + (g + 1) * GC].astype(BF16NP).reshape(1, GC)
        in_maps.append(im)

    try:
        res = run_bass_kernel_spmd(nc, in_maps, core_ids=list(range(NCORES)))
    except ModuleNotFoundError:
        # BASS_TRACE set but the NTFF profiling hook isn't available in this
        # environment -- rerun with tracing hard-disabled.
        os.environ["BASS_NEVER_TRACE"] = "1"
        res = run_bass_kernel_spmd(nc, in_maps, core_ids=list(range(NCORES)))
    kernel.last_results = res
    out = np.zeros((B, N, D), np.float32)
    for core in range(NCORES):
        b = core // GROUPS
        out[b] += np.asarray(res.results[core]["out"], dtype=np.float32)
    out += bo
    return out
